# revision 4
# baseline (speedup 1.0000x reference)
"""DeepSeekMoE (H=1024, I=4096, E=8, top-2, T=16384) on 8 Trainium2 cores.

Strategy (expert parallelism, per the sharding hint):
  - Host computes router softmax/top-2 (tiny: T x E) with jax-on-CPU so the
    expert selection matches the reference bit-for-bit.
  - Core i holds routed expert i's weights and processes the tokens routed
    to expert i (gathered+padded on the host: the "all-to-all" is done
    host-side since full inputs arrive on the host).
  - The shared expert is data-parallel: core i also runs tokens
    [i*T/8, (i+1)*T/8) through the (replicated) shared expert.
  - Load balance: rather than padding every core's routed batch to
    max_e(count_e), the routed capacity is R < max(count) and the overflow
    (token, expert) pairs are redistributed to a third small "V slot" on
    other cores, which receive that expert's weights as an extra input.
    Per-core tokens drop from 2048+max(c_e) to 2048+R+V with R+V chosen
    minimal such that the overflow chunks fit on the 8 cores.
  - Device computes MLPs in fp16 operands with fp32 PSUM accumulation in a
    transposed activation layout (hidden on partitions, tokens on the free
    dim), so no on-device transposes are needed anywhere.
  - fp8 DoubleRow slice: the first NF8 (of 32) intermediate-dim tiles of
    the down-projection run as fp8e4 DoubleRow matmuls (2 contraction
    rows/cycle, measured 2.0x vs fp16 per 256-contraction). silu output for
    those tiles is written to fp8 directly by the scalar engine (scale 1.0)
    and w2 rows are pre-scaled by 256 into fp8 range on the host; the whole
    PSUM is then 256*y (the fp16 w2 rows are also pre-scaled by 256), and
    the host divides the output by 256. Measured end-to-end rel err ~1.6e-2
    at NF8=6 vs the 2e-2 budget; fp16-only error is 4.2e-4.
  - Host applies the top-2 routing weights and scatter-adds routed expert
    outputs back into token order (each token appears at most once per
    expert, so per-expert fancy-index += is collision-free).
"""

import hashlib
import json
import os
import shutil

import numpy as np

H = 1024
I = 4096
E = 8
TOPK = 2
NCORES = 8
T = 16384
TS = T // NCORES  # shared-expert tokens per core
N = 512  # token tile (moving dim / one PSUM bank of fp32)

NF8 = 6  # intermediate-dim 128-tiles (of 32) computed in fp8 DoubleRow
W2SC = 256.0  # w2 pre-scale so fp8 slice and fp16 rows share PSUM units

_NEFF_CACHE_DIR = os.path.join(
    os.path.expanduser("~"), ".cache", "bass_neff_cache"
)

_compiled = {}  # (R, V) -> finalized Bacc
_cache_installed = False


def _install_neff_cache():
    """Cache walrus NEFF output by bir.json hash so repeated runs of the
    identical device program skip the multi-minute neuronxcc compile."""
    global _cache_installed
    if _cache_installed:
        return
    _cache_installed = True
    try:
        import concourse.bass_utils as bass_utils
        import concourse.bass2jax as bass2jax

        orig = bass_utils.compile_bir_kernel

        def canonical_key(bir_bytes):
            # The BIR embeds source paths/linenos (debug_table entries and
            # per-object ant_debug blobs). Strip those so the cache key only
            # reflects program semantics.
            try:
                m = json.loads(bir_bytes)
                m["debug_table"] = None
                stack = [m]
                while stack:
                    o = stack.pop()
                    if isinstance(o, dict):
                        o.pop("ant_debug", None)
                        stack.extend(o.values())
                    elif isinstance(o, list):
                        stack.extend(o)
                canon = json.dumps(m, sort_keys=True).encode()
            except Exception:
                canon = bir_bytes
            return hashlib.sha256(canon).hexdigest()

        def cached(bir_json, tmpdir, neff_name="file.neff"):
            if isinstance(bir_json, str):
                bir_bytes = bir_json.encode()
            else:
                bir_bytes = bir_json
            key = canonical_key(bir_bytes)
            cpath = os.path.join(_NEFF_CACHE_DIR, key + ".neff")
            dst = os.path.join(tmpdir, neff_name)
            if os.path.isfile(cpath):
                shutil.copyfile(cpath, dst)
                return dst
            out = orig(bir_json, tmpdir, neff_name)
            try:
                os.makedirs(_NEFF_CACHE_DIR, exist_ok=True)
                tmp = cpath + ".tmp%d" % os.getpid()
                shutil.copyfile(out, tmp)
                os.replace(tmp, cpath)
            except OSError:
                pass
            return out

        bass_utils.compile_bir_kernel = cached
        bass2jax.compile_bir_kernel = cached
    except Exception:
        pass


def _build(R, V):
    """Build the per-core SPMD device program: shared (TS tokens), routed
    (R tokens), overflow (V tokens, own weight inputs)."""
    import concourse.mybir as mybir
    import concourse.tile as tile
    from concourse import bacc

    f8 = mybir.dt.float8e4
    f16 = mybir.dt.float16
    f32 = mybir.dt.float32
    silu = mybir.ActivationFunctionType.Silu
    DR = mybir.MatmulPerfMode.DoubleRow

    nc = bacc.Bacc(None, target_bir_lowering=False)

    KT = H // 128   # 8 k-tiles over hidden
    IC = I // 128   # 32 i-chunks over intermediate
    HC = H // 128   # 8 output chunks over hidden
    K8 = NF8 * 128  # fp8 slice of the intermediate dim
    NPAIR = NF8 // 2

    def io(name, ntok):
        x = nc.dram_tensor("x" + name, [H, ntok], f16, kind="ExternalInput")
        w1 = nc.dram_tensor("w1" + name, [H, I], f16, kind="ExternalInput")
        w2 = nc.dram_tensor("w2" + name, [I - K8, H], f16,
                            kind="ExternalInput")
        w28 = nc.dram_tensor("w28" + name, [K8, H], f8, kind="ExternalInput")
        y = nc.dram_tensor("y" + name, [H, ntok], f32, kind="ExternalOutput")
        return x, w1, w2, w28, y

    ios = [(io("s", TS), TS), (io("r", R), R)]
    if V:
        ios.append((io("v", V), V))

    with tile.TileContext(nc) as tc:
        with tc.tile_pool(name="wp", bufs=1) as wp, \
             tc.tile_pool(name="xp", bufs=2) as xp, \
             tc.tile_pool(name="hp", bufs=1) as hp, \
             tc.tile_pool(name="yp", bufs=3) as yp, \
             tc.tile_pool(name="pp", bufs=2, space="PSUM") as pp:

            def load_x(xT, t0, n):
                xt = xp.tile([128, KT, N], f16, tag="x")
                nc.sync.dma_start(
                    out=xt[:, :, :n],
                    in_=xT[:, t0:t0 + n].rearrange("(kt p) n -> p kt n", p=128),
                )
                return xt

            def mlp(xT, w1, w2, w28, yT, ntok, first=False):
                w1t = wp.tile([128, KT, I], f16, tag="w1")
                w1r_ap = w1.rearrange("(kt p) i -> p kt i", p=128)
                if first:
                    # tiny head stripe so the very first matmul group
                    # (ic=0, k=0..7) is gated on ~0.26MB not 2MB
                    nc.sync.dma_start(out=w1t[:, :, :128],
                                      in_=w1r_ap[:, :, :128])
                xt0 = load_x(xT, 0, min(N, ntok))
                # weights striped into ~1-2MB DMAs: spreads across DMA queues
                # and lets the first matmuls start early
                for g in range(8):
                    sl = slice(g * (I // 8) + (128 if first and g == 0 else 0),
                               (g + 1) * (I // 8))
                    nc.sync.dma_start(out=w1t[:, :, sl], in_=w1r_ap[:, :, sl])
                w2t = wp.tile([128, IC - NF8, H], f16, tag="w2")
                w2r_ap = w2.rearrange("(it p) h -> p it h", p=128)
                for g in range(8):
                    lo = g * (IC - NF8) // 8
                    hi = (g + 1) * (IC - NF8) // 8
                    nc.sync.dma_start(out=w2t[:, lo:hi, :],
                                      in_=w2r_ap[:, lo:hi, :])
                w28t = wp.tile([128, NF8, H], f8, tag="w28")
                nc.sync.dma_start(
                    out=w28t, in_=w28.rearrange("(it p) h -> p it h", p=128))
                for t0 in range(0, ntok, N):
                    n = min(N, ntok - t0)
                    xt = xt0 if t0 == 0 else load_x(xT, t0, n)
                    ht = hp.tile([128, IC - NF8, N], f16, tag="h")
                    h8t = hp.tile([128, NF8, N], f8, tag="h8")
                    for ic in range(IC):
                        ps = pp.tile([128, N], f32, tag="hp")
                        for k in range(KT):
                            nc.tensor.matmul(
                                ps[:, :n],
                                w1t[:, k, ic * 128:(ic + 1) * 128],
                                xt[:, k, :n],
                                start=(k == 0),
                                stop=(k == KT - 1),
                            )
                        if ic < NF8:
                            nc.scalar.activation(h8t[:, ic, :n], ps[:, :n],
                                                 silu)
                        else:
                            nc.scalar.activation(ht[:, ic - NF8, :n],
                                                 ps[:, :n], silu)
                    for hc in range(HC):
                        yps = pp.tile([128, N], f32, tag="yp")
                        csl = slice(hc * 128, (hc + 1) * 128)
                        for j in range(NPAIR):
                            nc.tensor.matmul(
                                yps[:, :n],
                                w28t[:, 2 * j:2 * j + 2, csl],
                                h8t[:, 2 * j:2 * j + 2, :n],
                                start=(j == 0),
                                stop=False,
                                perf_mode=DR,
                            )
                        for ic in range(IC - NF8):
                            nc.tensor.matmul(
                                yps[:, :n],
                                w2t[:, ic, csl],
                                ht[:, ic, :n],
                                start=False,
                                stop=(ic == IC - NF8 - 1),
                            )
                        yt = yp.tile([128, N], f32, tag="y")
                        nc.vector.tensor_copy(yt[:, :n], yps[:, :n])
                        nc.sync.dma_start(out=yT[csl, t0:t0 + n],
                                          in_=yt[:, :n])

            for idx, ((x, w1, w2, w28, y), ntok) in enumerate(ios):
                mlp(x, w1, w2, w28, y, ntok, first=(idx == 0))

    nc.finalize()
    return nc


def _get_nc(R, V):
    key = (R, V)
    nc = _compiled.get(key)
    if nc is None:
        nc = _build(R, V)
        _compiled[key] = nc
    return nc


def _plan_overflow(counts):
    """Pick routed capacity R and overflow-slot size V minimizing R+V such
    that the overflow chunks (<=V tokens each, one chunk per core) fit on
    NCORES cores. Returns (R, V, chunks) with chunks = [(expert, lo, hi)]."""
    cap = int(counts.max())
    best = (cap, 0)
    for R in range(cap, max(int(counts.min()), cap - 600) - 1, -1):
        ovf = np.maximum(counts - R, 0)
        if not ovf.any():
            continue
        mx = int(ovf.max())
        # min V such that sum(ceil(ovf/V)) <= NCORES (monotone in V)
        lo, hi = 1, mx
        while lo < hi:
            mid = (lo + hi) // 2
            if sum(-(-int(o) // mid) for o in ovf if o) <= NCORES:
                hi = mid
            else:
                lo = mid + 1
        V = lo
        if sum(-(-int(o) // V) for o in ovf if o) > NCORES:
            continue
        if R + V < best[0] + best[1]:
            best = (R, V)
    R, V = best
    chunks = []
    if V:
        for e in range(E):
            o = int(max(counts[e] - R, 0))
            lo = R
            while o > 0:
                take = min(o, V)
                chunks.append((e, lo, lo + take))
                lo += take
                o -= take
    assert len(chunks) <= NCORES
    return R, V, chunks


# test-harness knobs (ignored in normal use)
TRACE = False
LAST_RESULT = None


def kernel(hidden_states, w1_shared, w2_shared, w1_routed, w2_routed,
           w_router):
    import jax
    import ml_dtypes
    from concourse.bass_utils import run_bass_kernel_spmd

    _install_neff_cache()

    E4 = ml_dtypes.float8_e4m3
    K8 = NF8 * 128

    hidden_states = np.asarray(hidden_states, dtype=np.float32)
    w_router = np.asarray(w_router, dtype=np.float32)
    flat = np.ascontiguousarray(hidden_states.reshape(-1, H))

    # --- routing on host, bit-identical to the reference (jax on CPU) ---
    cpu = jax.devices("cpu")[0]
    with jax.default_device(cpu):
        jflat = jax.device_put(flat, cpu)
        jrouter = jax.device_put(w_router, cpu)
        logits = jflat @ jrouter
        rw = jax.nn.softmax(logits, axis=-1)
        topw, topi = jax.lax.top_k(rw, TOPK)
        topw = topw / jax.numpy.sum(topw, axis=-1, keepdims=True)
    topw = np.asarray(topw)  # [T, K] f32
    topi = np.asarray(topi)  # [T, K] int32

    pairs_e = topi.ravel()  # expert of each (token, k) slot
    order = np.argsort(pairs_e, kind="stable")
    counts = np.bincount(pairs_e, minlength=E)
    starts = np.zeros(E + 1, np.int64)
    np.cumsum(counts, out=starts[1:])
    tok_by_e = [order[starts[e]:starts[e + 1]] // TOPK for e in range(E)]
    w_by_e = [topw.ravel()[order[starts[e]:starts[e + 1]]] for e in range(E)]

    R, V, chunks = _plan_overflow(counts)

    # --- build per-core inputs (fp16 / fp8, transposed activations) ---
    flatT16 = np.ascontiguousarray(flat.T.astype(np.float16))  # [H, T]
    w1s16 = np.asarray(w1_shared, dtype=np.float16)
    w1r16 = np.asarray(w1_routed, dtype=np.float16)

    def w2_prep(w2):
        w2 = np.asarray(w2, np.float32) * W2SC
        return (np.ascontiguousarray(w2[K8:]).astype(np.float16),
                np.ascontiguousarray(np.clip(w2[:K8], -240, 240)).astype(E4))

    w2s16, w2s8 = w2_prep(w2_shared)
    w2r = [w2_prep(np.asarray(w2_routed[e], np.float32)) for e in range(E)]

    zero_w1 = np.zeros((H, I), np.float16)
    zero_w2 = np.zeros((I - K8, H), np.float16)
    zero_w28 = np.zeros((K8, H), E4)

    in_maps = []
    for i in range(NCORES):
        xr_i = np.zeros((H, R), np.float16)
        nr = min(int(counts[i]), R)
        xr_i[:, :nr] = flatT16[:, tok_by_e[i][:nr]]
        m = {
            "xs": np.ascontiguousarray(flatT16[:, i * TS:(i + 1) * TS]),
            "xr": xr_i,
            "w1s": w1s16,
            "w2s": w2s16,
            "w28s": w2s8,
            "w1r": w1r16[i],
            "w2r": w2r[i][0],
            "w28r": w2r[i][1],
        }
        if V:
            if i < len(chunks):
                e, lo, hi = chunks[i]
                xv_i = np.zeros((H, V), np.float16)
                xv_i[:, :hi - lo] = flatT16[:, tok_by_e[e][lo:hi]]
                m["xv"] = xv_i
                m["w1v"] = w1r16[e]
                m["w2v"] = w2r[e][0]
                m["w28v"] = w2r[e][1]
            else:
                m["xv"] = np.zeros((H, V), np.float16)
                m["w1v"] = zero_w1
                m["w2v"] = zero_w2
                m["w28v"] = zero_w28
        in_maps.append(m)

    nc = _get_nc(R, V)
    try:
        res = run_bass_kernel_spmd(nc, in_maps, list(range(NCORES)),
                                   trace=TRACE)
    except Exception:
        # transient NRT/device hiccups have been observed to clear on retry
        res = run_bass_kernel_spmd(nc, in_maps, list(range(NCORES)),
                                   trace=TRACE)
    global LAST_RESULT
    LAST_RESULT = res

    # --- combine on host ---
    inv = np.float32(1.0 / W2SC)
    total = np.empty((T, H), np.float32)
    for i in range(NCORES):
        total[i * TS:(i + 1) * TS] = res.results[i]["ys"].T
    total *= inv
    routed = np.zeros((T, H), np.float32)
    for e in range(E):
        ne = min(int(counts[e]), R)
        if ne:
            ye = res.results[e]["yr"][:, :ne].T * inv  # [ne, H]
            routed[tok_by_e[e][:ne]] += w_by_e[e][:ne, None] * ye
    for i, (e, lo, hi) in enumerate(chunks):
        yv = res.results[i]["yv"][:, :hi - lo].T * inv
        routed[tok_by_e[e][lo:hi]] += w_by_e[e][lo:hi, None] * yv
    total += routed
    return total.reshape(hidden_states.shape)


# revision 8
# speedup vs baseline: 1.0211x; 1.0211x over previous
"""DeepSeekMoE (H=1024, I=4096, E=8, top-2, T=16384) on 8 Trainium2 cores.

Strategy (expert parallelism, per the sharding hint):
  - Host computes router softmax/top-2 (tiny: T x E) with jax-on-CPU so the
    expert selection matches the reference bit-for-bit.
  - Core i holds routed expert i's weights and processes the tokens routed
    to expert i (gathered+padded on the host: the "all-to-all" is done
    host-side since full inputs arrive on the host).
  - The shared expert is data-parallel: core i also runs tokens
    [i*T/8, (i+1)*T/8) through the (replicated) shared expert.
  - Load balance: rather than padding every core's routed batch to
    max_e(count_e), the routed capacity is R < max(count) and the overflow
    (token, expert) pairs are redistributed to a third small "V slot" on
    other cores, which receive that expert's weights as an extra input.
    Per-core tokens drop from 2048+max(c_e) to 2048+R+V with R+V chosen
    minimal such that the overflow chunks fit on the 8 cores.
  - Device computes MLPs in fp16 operands with fp32 PSUM accumulation in a
    transposed activation layout (hidden on partitions, tokens on the free
    dim), so no on-device transposes are needed anywhere.
  - fp8 DoubleRow slice: the first NF8 (of 32) intermediate-dim tiles of
    the down-projection run as fp8e4 DoubleRow matmuls (2 contraction
    rows/cycle, measured 2.0x vs fp16 per 256-contraction). silu output for
    those tiles is written to fp8 directly by the scalar engine (scale 1.0)
    and w2 rows are pre-scaled by 256 into fp8 range on the host; the whole
    PSUM is then 256*y (the fp16 w2 rows are also pre-scaled by 256), and
    the host divides the output by 256. Measured end-to-end rel err ~1.6e-2
    at NF8=6 vs the 2e-2 budget; fp16-only error is 4.2e-4.
  - Host applies the top-2 routing weights and scatter-adds routed expert
    outputs back into token order (each token appears at most once per
    expert, so per-expert fancy-index += is collision-free).
"""

import hashlib
import json
import os
import shutil

import numpy as np

H = 1024
I = 4096
E = 8
TOPK = 2
NCORES = 8
T = 16384
TS = T // NCORES  # shared-expert tokens per core
N = 512  # token tile (moving dim / one PSUM bank of fp32)

NF8 = 8  # intermediate-dim 128-tiles (of 32) computed in fp8 DoubleRow
W2SC = 256.0  # w2 pre-scale so the fp8 slice stays in fp8-friendly range

_NEFF_CACHE_DIR = os.path.join(
    os.path.expanduser("~"), ".cache", "bass_neff_cache"
)

_compiled = {}  # (R, V) -> finalized Bacc
_cache_installed = False


def _install_neff_cache():
    """Cache walrus NEFF output by bir.json hash so repeated runs of the
    identical device program skip the multi-minute neuronxcc compile."""
    global _cache_installed
    if _cache_installed:
        return
    _cache_installed = True
    try:
        import concourse.bass_utils as bass_utils
        import concourse.bass2jax as bass2jax

        orig = bass_utils.compile_bir_kernel

        def canonical_key(bir_bytes):
            # The BIR embeds source paths/linenos (debug_table entries and
            # per-object ant_debug blobs). Strip those so the cache key only
            # reflects program semantics.
            try:
                m = json.loads(bir_bytes)
                m["debug_table"] = None
                stack = [m]
                while stack:
                    o = stack.pop()
                    if isinstance(o, dict):
                        o.pop("ant_debug", None)
                        stack.extend(o.values())
                    elif isinstance(o, list):
                        stack.extend(o)
                canon = json.dumps(m, sort_keys=True).encode()
            except Exception:
                canon = bir_bytes
            return hashlib.sha256(canon).hexdigest()

        def cached(bir_json, tmpdir, neff_name="file.neff"):
            if isinstance(bir_json, str):
                bir_bytes = bir_json.encode()
            else:
                bir_bytes = bir_json
            key = canonical_key(bir_bytes)
            cpath = os.path.join(_NEFF_CACHE_DIR, key + ".neff")
            dst = os.path.join(tmpdir, neff_name)
            if os.path.isfile(cpath):
                shutil.copyfile(cpath, dst)
                return dst
            out = orig(bir_json, tmpdir, neff_name)
            try:
                os.makedirs(_NEFF_CACHE_DIR, exist_ok=True)
                tmp = cpath + ".tmp%d" % os.getpid()
                shutil.copyfile(out, tmp)
                os.replace(tmp, cpath)
            except OSError:
                pass
            return out

        bass_utils.compile_bir_kernel = cached
        bass2jax.compile_bir_kernel = cached
    except Exception:
        pass


def _build(R, V):
    """Build the per-core SPMD device program: shared (TS tokens), routed
    (R tokens), overflow (V tokens, own weight inputs)."""
    import concourse.mybir as mybir
    import concourse.tile as tile
    from concourse import bacc

    f8 = mybir.dt.float8e4
    f16 = mybir.dt.float16
    f32 = mybir.dt.float32
    silu = mybir.ActivationFunctionType.Silu
    DR = mybir.MatmulPerfMode.DoubleRow

    nc = bacc.Bacc(None, target_bir_lowering=False)

    KT = H // 128   # 8 k-tiles over hidden
    IC = I // 128   # 32 i-chunks over intermediate
    HC = H // 128   # 8 output chunks over hidden
    K8 = NF8 * 128  # fp8 slice of the intermediate dim
    NPAIR = NF8 // 2

    def io(name, ntok):
        x = nc.dram_tensor("x" + name, [H, ntok], f16, kind="ExternalInput")
        w1 = nc.dram_tensor("w1" + name, [H, I], f16, kind="ExternalInput")
        w2 = nc.dram_tensor("w2" + name, [I - K8, H], f16,
                            kind="ExternalInput")
        w28 = nc.dram_tensor("w28" + name, [K8, H], f8, kind="ExternalInput")
        y = nc.dram_tensor("y" + name, [H, ntok], f32, kind="ExternalOutput")
        # transposed fp8-slice contribution, added on the host
        y8 = nc.dram_tensor("y8" + name, [ntok, H], f16,
                            kind="ExternalOutput")
        return x, w1, w2, w28, y, y8

    ios = [(io("s", TS), TS), (io("r", R), R)]
    if V:
        ios.append((io("v", V), V))

    with tile.TileContext(nc) as tc:
        with tc.tile_pool(name="wp", bufs=1) as wp, \
             tc.tile_pool(name="xp", bufs=3) as xp, \
             tc.tile_pool(name="hp", bufs=1) as hp, \
             tc.tile_pool(name="h8p", bufs=2) as h8p, \
             tc.tile_pool(name="yp", bufs=3) as yp, \
             tc.tile_pool(name="pp", bufs=2, space="PSUM") as pp, \
             tc.tile_pool(name="p8", bufs=2, space="PSUM") as p8:

            def load_x(xT, t0, n):
                xt = xp.tile([128, KT, N], f16, tag="x")
                nc.sync.dma_start(
                    out=xt[:, :, :n],
                    in_=xT[:, t0:t0 + n].rearrange("(kt p) n -> p kt n", p=128),
                )
                return xt

            def mlp(xT, w1, w2, w28, yT, y8T, ntok, first=False):
                w1t = wp.tile([128, KT, I], f16, tag="w1")
                w1r_ap = w1.rearrange("(kt p) i -> p kt i", p=128)
                if first:
                    # tiny head stripe so the very first matmul group
                    # (first ic, k=0..7) is gated on ~0.26MB not 2MB
                    hd = slice(NF8 * 128, NF8 * 128 + 128)
                    nc.sync.dma_start(out=w1t[:, :, hd], in_=w1r_ap[:, :, hd])
                xt0 = load_x(xT, 0, min(N, ntok))
                # small fp8 w2 slice early: first DR block needs it well
                # before the bulky fp16 stripes finish
                w28t = wp.tile([128, NF8, H], f8, tag="w28")
                nc.sync.dma_start(
                    out=w28t, in_=w28.rearrange("(it p) h -> p it h", p=128))
                # weights striped into ~1-2MB DMAs: spreads across DMA queues
                # and lets the first matmuls start early
                for g in range(8):
                    lo = g * (I // 8)
                    if first and lo == NF8 * 128:
                        lo += 128  # head stripe already loaded this range
                    sl = slice(lo, (g + 1) * (I // 8))
                    nc.sync.dma_start(out=w1t[:, :, sl], in_=w1r_ap[:, :, sl])
                w2t = wp.tile([128, IC - NF8, H], f16, tag="w2")
                w2r_ap = w2.rearrange("(it p) h -> p it h", p=128)
                for g in range(8):
                    lo = g * (IC - NF8) // 8
                    hi = (g + 1) * (IC - NF8) // 8
                    nc.sync.dma_start(out=w2t[:, lo:hi, :],
                                      in_=w2r_ap[:, lo:hi, :])

                def dr_block(h8t, t0, n):
                    # fp8 slice, swapped operands: h8 pair chunks stationary,
                    # w2 rows moving; output y8[t, h] accumulated per chunk
                    for c in range(0, n, 128):
                        rem = min(128, n - c)
                        pa = p8.tile([128, 512], f32, tag="y8a")
                        pb = p8.tile([128, 512], f32, tag="y8b")
                        for j in range(NPAIR):
                            st = h8t[:, 2 * j:2 * j + 2, c:c + rem]
                            nc.tensor.matmul(
                                pa[:rem, :], st, w28t[:, 2 * j:2 * j + 2,
                                                      0:512],
                                start=(j == 0), stop=(j == NPAIR - 1),
                                perf_mode=DR)
                            nc.tensor.matmul(
                                pb[:rem, :], st, w28t[:, 2 * j:2 * j + 2,
                                                      512:1024],
                                start=(j == 0), stop=(j == NPAIR - 1),
                                perf_mode=DR)
                        y8t = yp.tile([128, H], f16, tag="y8s")
                        nc.vector.tensor_copy(y8t[:rem, 0:512], pa[:rem, :])
                        nc.vector.tensor_copy(y8t[:rem, 512:1024],
                                              pb[:rem, :])
                        nc.sync.dma_start(out=y8T[t0 + c:t0 + c + rem, :],
                                          in_=y8t[:rem, :])

                pend = []
                for t0 in range(0, ntok, N):
                    n = min(N, ntok - t0)
                    xt = xt0 if t0 == 0 else load_x(xT, t0, n)
                    ht = hp.tile([128, IC - NF8, N], f16, tag="h")
                    h8t = h8p.tile([128, NF8, N], f8, tag="h8")
                    # fp16 ic-tiles first so stage-2 isn't gated on the
                    # last activation; fp8 tiles (read later) last
                    for ic in list(range(NF8, IC)) + list(range(NF8)):
                        ps = pp.tile([128, N], f32, tag="hp")
                        for k in range(KT):
                            nc.tensor.matmul(
                                ps[:, :n],
                                w1t[:, k, ic * 128:(ic + 1) * 128],
                                xt[:, k, :n],
                                start=(k == 0),
                                stop=(k == KT - 1),
                            )
                        if ic < NF8:
                            nc.scalar.activation(h8t[:, ic, :n], ps[:, :n],
                                                 silu)
                        else:
                            nc.scalar.activation(ht[:, ic - NF8, :n],
                                                 ps[:, :n], silu)
                    for hc in range(HC):
                        yps = pp.tile([128, N], f32, tag="yp")
                        csl = slice(hc * 128, (hc + 1) * 128)
                        for ic in range(IC - NF8):
                            nc.tensor.matmul(
                                yps[:, :n],
                                w2t[:, ic, csl],
                                ht[:, ic, :n],
                                start=(ic == 0),
                                stop=(ic == IC - NF8 - 1),
                            )
                        yt = yp.tile([128, N], f32, tag="y")
                        nc.vector.tensor_copy(yt[:, :n], yps[:, :n])
                        nc.sync.dma_start(out=yT[csl, t0:t0 + n],
                                          in_=yt[:, :n])
                    pend.append((h8t, t0, n))
                    if len(pend) == 2 or t0 + N >= ntok:
                        for args in pend:
                            dr_block(*args)
                        pend = []

            for idx, ((x, w1, w2, w28, y, y8), ntok) in enumerate(ios):
                mlp(x, w1, w2, w28, y, y8, ntok, first=(idx == 0))

    nc.finalize()
    return nc


def _get_nc(R, V):
    key = (R, V)
    nc = _compiled.get(key)
    if nc is None:
        nc = _build(R, V)
        _compiled[key] = nc
    return nc


def _plan_overflow(counts):
    """Pick routed capacity R and overflow-slot size V minimizing R+V such
    that the overflow chunks (<=V tokens each, one chunk per core) fit on
    NCORES cores. Returns (R, V, chunks) with chunks = [(expert, lo, hi)]."""
    cap = int(counts.max())
    best = (cap, 0)
    for R in range(cap, max(int(counts.min()), cap - 600) - 1, -1):
        ovf = np.maximum(counts - R, 0)
        if not ovf.any():
            continue
        mx = int(ovf.max())
        # min V such that sum(ceil(ovf/V)) <= NCORES (monotone in V)
        lo, hi = 1, mx
        while lo < hi:
            mid = (lo + hi) // 2
            if sum(-(-int(o) // mid) for o in ovf if o) <= NCORES:
                hi = mid
            else:
                lo = mid + 1
        V = lo
        if sum(-(-int(o) // V) for o in ovf if o) > NCORES:
            continue
        if R + V < best[0] + best[1]:
            best = (R, V)
    R, V = best
    chunks = []
    if V:
        for e in range(E):
            o = int(max(counts[e] - R, 0))
            lo = R
            while o > 0:
                take = min(o, V)
                chunks.append((e, lo, lo + take))
                lo += take
                o -= take
    assert len(chunks) <= NCORES
    return R, V, chunks


# test-harness knobs (ignored in normal use)
TRACE = False
LAST_RESULT = None


def kernel(hidden_states, w1_shared, w2_shared, w1_routed, w2_routed,
           w_router):
    import jax
    import ml_dtypes
    from concourse.bass_utils import run_bass_kernel_spmd

    _install_neff_cache()

    E4 = ml_dtypes.float8_e4m3
    K8 = NF8 * 128

    hidden_states = np.asarray(hidden_states, dtype=np.float32)
    w_router = np.asarray(w_router, dtype=np.float32)
    flat = np.ascontiguousarray(hidden_states.reshape(-1, H))

    # --- routing on host, bit-identical to the reference (jax on CPU) ---
    cpu = jax.devices("cpu")[0]
    with jax.default_device(cpu):
        jflat = jax.device_put(flat, cpu)
        jrouter = jax.device_put(w_router, cpu)
        logits = jflat @ jrouter
        rw = jax.nn.softmax(logits, axis=-1)
        topw, topi = jax.lax.top_k(rw, TOPK)
        topw = topw / jax.numpy.sum(topw, axis=-1, keepdims=True)
    topw = np.asarray(topw)  # [T, K] f32
    topi = np.asarray(topi)  # [T, K] int32

    pairs_e = topi.ravel()  # expert of each (token, k) slot
    order = np.argsort(pairs_e, kind="stable")
    counts = np.bincount(pairs_e, minlength=E)
    starts = np.zeros(E + 1, np.int64)
    np.cumsum(counts, out=starts[1:])
    tok_by_e = [order[starts[e]:starts[e + 1]] // TOPK for e in range(E)]
    w_by_e = [topw.ravel()[order[starts[e]:starts[e + 1]]] for e in range(E)]

    R, V, chunks = _plan_overflow(counts)

    # --- build per-core inputs (fp16 / fp8, transposed activations) ---
    flatT16 = np.ascontiguousarray(flat.T.astype(np.float16))  # [H, T]
    w1s16 = np.asarray(w1_shared, dtype=np.float16)
    w1r16 = np.asarray(w1_routed, dtype=np.float16)

    def w2_prep(w2):
        w2 = np.asarray(w2, np.float32) * W2SC
        return (np.ascontiguousarray(w2[K8:]).astype(np.float16),
                np.ascontiguousarray(np.clip(w2[:K8], -240, 240)).astype(E4))

    w2s16, w2s8 = w2_prep(w2_shared)
    w2r = [w2_prep(np.asarray(w2_routed[e], np.float32)) for e in range(E)]

    zero_w1 = np.zeros((H, I), np.float16)
    zero_w2 = np.zeros((I - K8, H), np.float16)
    zero_w28 = np.zeros((K8, H), E4)

    in_maps = []
    for i in range(NCORES):
        xr_i = np.zeros((H, R), np.float16)
        nr = min(int(counts[i]), R)
        xr_i[:, :nr] = flatT16[:, tok_by_e[i][:nr]]
        m = {
            "xs": np.ascontiguousarray(flatT16[:, i * TS:(i + 1) * TS]),
            "xr": xr_i,
            "w1s": w1s16,
            "w2s": w2s16,
            "w28s": w2s8,
            "w1r": w1r16[i],
            "w2r": w2r[i][0],
            "w28r": w2r[i][1],
        }
        if V:
            if i < len(chunks):
                e, lo, hi = chunks[i]
                xv_i = np.zeros((H, V), np.float16)
                xv_i[:, :hi - lo] = flatT16[:, tok_by_e[e][lo:hi]]
                m["xv"] = xv_i
                m["w1v"] = w1r16[e]
                m["w2v"] = w2r[e][0]
                m["w28v"] = w2r[e][1]
            else:
                m["xv"] = np.zeros((H, V), np.float16)
                m["w1v"] = zero_w1
                m["w2v"] = zero_w2
                m["w28v"] = zero_w28
        in_maps.append(m)

    nc = _get_nc(R, V)
    try:
        res = run_bass_kernel_spmd(nc, in_maps, list(range(NCORES)),
                                   trace=TRACE)
    except Exception:
        # transient NRT/device hiccups have been observed to clear on retry
        res = run_bass_kernel_spmd(nc, in_maps, list(range(NCORES)),
                                   trace=TRACE)
    global LAST_RESULT
    LAST_RESULT = res

    # --- combine on host ---
    inv = np.float32(1.0 / W2SC)
    total = np.empty((T, H), np.float32)
    for i in range(NCORES):
        r = res.results[i]
        total[i * TS:(i + 1) * TS] = r["ys"].T + r["y8s"].astype(np.float32)
    total *= inv
    routed = np.zeros((T, H), np.float32)
    for e in range(E):
        ne = min(int(counts[e]), R)
        if ne:
            r = res.results[e]
            ye = (r["yr"][:, :ne].T
                  + r["y8r"][:ne].astype(np.float32)) * inv  # [ne, H]
            routed[tok_by_e[e][:ne]] += w_by_e[e][:ne, None] * ye
    for i, (e, lo, hi) in enumerate(chunks):
        r = res.results[i]
        yv = (r["yv"][:, :hi - lo].T
              + r["y8v"][:hi - lo].astype(np.float32)) * inv
        routed[tok_by_e[e][lo:hi]] += w_by_e[e][lo:hi, None] * yv
    total += routed
    return total.reshape(hidden_states.shape)


# revision 14
# speedup vs baseline: 1.0487x; 1.0270x over previous
"""DeepSeekMoE (H=1024, I=4096, E=8, top-2, T=16384) on 8 Trainium2 cores.

Strategy (expert parallelism, per the sharding hint):
  - Host computes router softmax/top-2 (tiny: T x E) with jax-on-CPU so the
    expert selection matches the reference bit-for-bit.
  - Core i holds routed expert i's weights and processes the tokens routed
    to expert i (gathered+padded on the host: the "all-to-all" is done
    host-side since full inputs arrive on the host).
  - The shared expert is data-parallel: core i also runs tokens
    [i*T/8, (i+1)*T/8) through the (replicated) shared expert.
  - Load balance: rather than padding every core's routed batch to
    max_e(count_e), the routed capacity is R < max(count) and the overflow
    (token, expert) pairs are redistributed to a third small "V slot" on
    other cores, which receive that expert's weights as an extra input.
    Per-core tokens drop from 2048+max(c_e) to 2048+R+V with R+V chosen
    minimal such that the overflow chunks fit on the 8 cores.
  - Device computes MLPs in fp16 operands with fp32 PSUM accumulation in a
    transposed activation layout (hidden on partitions, tokens on the free
    dim), so no on-device transposes are needed anywhere.
  - fp8 DoubleRow slice: the first NF8 (of 32) intermediate-dim tiles of
    the down-projection run as fp8e4 DoubleRow matmuls (2 contraction
    rows/cycle, measured 2.0x vs fp16 per 256-contraction). silu output for
    those tiles is written to fp8 directly by the scalar engine (scale 1.0)
    and w2 rows are pre-scaled by 256 into fp8 range on the host; the whole
    PSUM is then 256*y (the fp16 w2 rows are also pre-scaled by 256), and
    the host divides the output by 256. Measured end-to-end rel err ~1.6e-2
    at NF8=6 vs the 2e-2 budget; fp16-only error is 4.2e-4.
  - Host applies the top-2 routing weights and scatter-adds routed expert
    outputs back into token order (each token appears at most once per
    expert, so per-expert fancy-index += is collision-free).
"""

import hashlib
import json
import os
import shutil

import numpy as np

H = 1024
I = 4096
E = 8
TOPK = 2
NCORES = 8
T = 16384
TS = T // NCORES  # shared-expert tokens per core
N = 512  # token tile (moving dim / one PSUM bank of fp32)

NF8 = 8  # intermediate-dim 128-tiles (of 32) computed in fp8 DoubleRow
W2SC = 256.0  # w2 pre-scale so the fp8 slice stays in fp8-friendly range

_NEFF_CACHE_DIR = os.path.join(
    os.path.expanduser("~"), ".cache", "bass_neff_cache"
)

_compiled = {}  # (R, V) -> finalized Bacc
_cache_installed = False


def _install_neff_cache():
    """Cache walrus NEFF output by bir.json hash so repeated runs of the
    identical device program skip the multi-minute neuronxcc compile."""
    global _cache_installed
    if _cache_installed:
        return
    _cache_installed = True
    try:
        import concourse.bass_utils as bass_utils
        import concourse.bass2jax as bass2jax

        orig = bass_utils.compile_bir_kernel

        def canonical_key(bir_bytes):
            # The BIR embeds source paths/linenos (debug_table entries and
            # per-object ant_debug blobs). Strip those so the cache key only
            # reflects program semantics.
            try:
                m = json.loads(bir_bytes)
                m["debug_table"] = None
                stack = [m]
                while stack:
                    o = stack.pop()
                    if isinstance(o, dict):
                        o.pop("ant_debug", None)
                        stack.extend(o.values())
                    elif isinstance(o, list):
                        stack.extend(o)
                canon = json.dumps(m, sort_keys=True).encode()
            except Exception:
                canon = bir_bytes
            return hashlib.sha256(canon).hexdigest()

        def cached(bir_json, tmpdir, neff_name="file.neff"):
            if isinstance(bir_json, str):
                bir_bytes = bir_json.encode()
            else:
                bir_bytes = bir_json
            key = canonical_key(bir_bytes)
            cpath = os.path.join(_NEFF_CACHE_DIR, key + ".neff")
            dst = os.path.join(tmpdir, neff_name)
            if os.path.isfile(cpath):
                shutil.copyfile(cpath, dst)
                return dst
            out = orig(bir_json, tmpdir, neff_name)
            try:
                os.makedirs(_NEFF_CACHE_DIR, exist_ok=True)
                tmp = cpath + ".tmp%d" % os.getpid()
                shutil.copyfile(out, tmp)
                os.replace(tmp, cpath)
            except OSError:
                pass
            return out

        bass_utils.compile_bir_kernel = cached
        bass2jax.compile_bir_kernel = cached
    except Exception:
        pass


def _build(R, V):
    """Build the per-core SPMD device program: shared (TS tokens), routed
    (R tokens), overflow (V tokens, own weight inputs)."""
    import concourse.mybir as mybir
    import concourse.tile as tile
    from concourse import bacc

    f8 = mybir.dt.float8e4
    f16 = mybir.dt.float16
    f32 = mybir.dt.float32
    silu = mybir.ActivationFunctionType.Silu
    copy_fn = mybir.ActivationFunctionType.Copy
    DR = mybir.MatmulPerfMode.DoubleRow

    nc = bacc.Bacc(None, target_bir_lowering=False)

    KT = H // 128   # 8 k-tiles over hidden
    IC = I // 128   # 32 i-chunks over intermediate
    HC = H // 128   # 8 output chunks over hidden
    K8 = NF8 * 128  # fp8 slice of the intermediate dim
    NPAIR = NF8 // 2

    def io(name, ntok):
        x = nc.dram_tensor("x" + name, [H, ntok], f16, kind="ExternalInput")
        w1 = nc.dram_tensor("w1" + name, [H, I], f16, kind="ExternalInput")
        w2 = nc.dram_tensor("w2" + name, [I - K8, H], f16,
                            kind="ExternalInput")
        w28 = nc.dram_tensor("w28" + name, [K8, H], f8, kind="ExternalInput")
        y = nc.dram_tensor("y" + name, [H, ntok], f32, kind="ExternalOutput")
        # transposed fp8-slice contribution, added on the host
        y8 = nc.dram_tensor("y8" + name, [ntok, H], f16,
                            kind="ExternalOutput")
        return x, w1, w2, w28, y, y8

    ios = [(io("s", TS), TS), (io("r", R), R)]
    if V:
        ios.append((io("v", V), V))

    with tile.TileContext(nc) as tc:
        with tc.tile_pool(name="wp", bufs=1) as wp, \
             tc.tile_pool(name="xp", bufs=3) as xp, \
             tc.tile_pool(name="hp", bufs=1) as hp, \
             tc.tile_pool(name="h8p", bufs=2) as h8p, \
             tc.tile_pool(name="yp", bufs=3) as yp, \
             tc.tile_pool(name="pp", bufs=3, space="PSUM") as pp:

            def load_x(xT, t0, n):
                xt = xp.tile([128, KT, N], f16, tag="x")
                nc.sync.dma_start(
                    out=xt[:, :, :n],
                    in_=xT[:, t0:t0 + n].rearrange("(kt p) n -> p kt n", p=128),
                )
                return xt

            def mlp(xT, w1, w2, w28, yT, y8T, ntok, first=False):
                xt0 = load_x(xT, 0, min(N, ntok))
                # small fp8 w2 slice early so the first DR block is never
                # gated on it behind the bulky fp16 stripes
                w28t = wp.tile([128, NF8, H], f8, tag="w28")
                nc.sync.dma_start(
                    out=w28t, in_=w28.rearrange("(it p) h -> p it h", p=128))
                # weights striped into ~1MB DMAs: spreads across DMA queues
                # and lets the first matmuls start early
                w1t = wp.tile([128, KT, I], f16, tag="w1")
                w1r_ap = w1.rearrange("(kt p) i -> p kt i", p=128)
                for g in range(16):
                    sl = slice(g * (I // 16), (g + 1) * (I // 16))
                    nc.sync.dma_start(out=w1t[:, :, sl], in_=w1r_ap[:, :, sl])
                w2t = wp.tile([128, IC - NF8, H], f16, tag="w2")
                w2r_ap = w2.rearrange("(it p) h -> p it h", p=128)
                for g in range(8):
                    lo = g * (IC - NF8) // 8
                    hi = (g + 1) * (IC - NF8) // 8
                    nc.sync.dma_start(out=w2t[:, lo:hi, :],
                                      in_=w2r_ap[:, lo:hi, :])

                def dr_block(h8t, t0, n):
                    # fp8 slice, swapped operands: h8 pair chunks stationary,
                    # w2 rows moving; output y8[t, h] accumulated per chunk.
                    # pa/pb sequential so each stationary pair loads once per
                    # half; 1:1 LDW:MM pipelines at full rate in DR streams.
                    for c in range(0, n, 128):
                        rem = min(128, n - c)
                        pa = pp.tile([128, N], f32, tag="mm")
                        for j in range(NPAIR):
                            nc.tensor.matmul(
                                pa[:rem, :],
                                h8t[:, 2 * j:2 * j + 2, c:c + rem],
                                w28t[:, 2 * j:2 * j + 2, 0:512],
                                start=(j == 0), stop=(j == NPAIR - 1),
                                perf_mode=DR)
                        pb = pp.tile([128, N], f32, tag="mm")
                        for j in range(NPAIR):
                            nc.tensor.matmul(
                                pb[:rem, :],
                                h8t[:, 2 * j:2 * j + 2, c:c + rem],
                                w28t[:, 2 * j:2 * j + 2, 512:1024],
                                start=(j == 0), stop=(j == NPAIR - 1),
                                perf_mode=DR)
                        y8t = yp.tile([128, H], f16, tag="y8s")
                        nc.scalar.activation(y8t[:rem, 0:512], pa[:rem, :],
                                             copy_fn)
                        nc.scalar.activation(y8t[:rem, 512:1024], pb[:rem, :],
                                             copy_fn)
                        nc.sync.dma_start(out=y8T[t0 + c:t0 + c + rem, :],
                                          in_=y8t[:rem, :])

                pend = []
                for t0 in range(0, ntok, N):
                    n = min(N, ntok - t0)
                    xt = xt0 if t0 == 0 else load_x(xT, t0, n)
                    ht = hp.tile([128, IC - NF8, N], f16, tag="h")
                    h8t = h8p.tile([128, NF8, N], f8, tag="h8")
                    # fp16 ic-tiles first so stage-2 isn't gated on the
                    # last activation; fp8 tiles (read later) last
                    for ic in list(range(NF8, IC)) + list(range(NF8)):
                        ps = pp.tile([128, N], f32, tag="mm")
                        for k in range(KT):
                            nc.tensor.matmul(
                                ps[:, :n],
                                w1t[:, k, ic * 128:(ic + 1) * 128],
                                xt[:, k, :n],
                                start=(k == 0),
                                stop=(k == KT - 1),
                            )
                        if ic < NF8:
                            nc.scalar.activation(h8t[:, ic, :n], ps[:, :n],
                                                 silu)
                        else:
                            nc.scalar.activation(ht[:, ic - NF8, :n],
                                                 ps[:, :n], silu)
                    for hc in range(HC):
                        yps = pp.tile([128, N], f32, tag="mm")
                        csl = slice(hc * 128, (hc + 1) * 128)
                        for ic in range(IC - NF8):
                            nc.tensor.matmul(
                                yps[:, :n],
                                w2t[:, ic, csl],
                                ht[:, ic, :n],
                                start=(ic == 0),
                                stop=(ic == IC - NF8 - 1),
                            )
                        yt = yp.tile([128, N], f32, tag="y")
                        nc.vector.tensor_copy(yt[:, :n], yps[:, :n])
                        nc.sync.dma_start(out=yT[csl, t0:t0 + n],
                                          in_=yt[:, :n])
                    pend.append((h8t, t0, n))
                    if len(pend) == 2 or t0 + N >= ntok:
                        for args in pend:
                            dr_block(*args)
                        pend = []

            for idx, ((x, w1, w2, w28, y, y8), ntok) in enumerate(ios):
                mlp(x, w1, w2, w28, y, y8, ntok, first=(idx == 0))

    nc.finalize()
    return nc


def _get_nc(R, V):
    key = (R, V)
    nc = _compiled.get(key)
    if nc is None:
        nc = _build(R, V)
        _compiled[key] = nc
    return nc


def _plan_overflow(counts):
    """Routed capacity per core. A separate overflow slot was measured to be
    a wash: its ~16MB of weights can't hide behind ~20us of compute at the
    kernel tail, so every core just pads to the max expert count."""
    return int(counts.max()), 0, []


# test-harness knobs (ignored in normal use)
TRACE = False
LAST_RESULT = None


def kernel(hidden_states, w1_shared, w2_shared, w1_routed, w2_routed,
           w_router):
    import jax
    import ml_dtypes
    from concourse.bass_utils import run_bass_kernel_spmd

    _install_neff_cache()

    E4 = ml_dtypes.float8_e4m3
    K8 = NF8 * 128

    hidden_states = np.asarray(hidden_states, dtype=np.float32)
    w_router = np.asarray(w_router, dtype=np.float32)
    flat = np.ascontiguousarray(hidden_states.reshape(-1, H))

    # --- routing on host, bit-identical to the reference (jax on CPU) ---
    cpu = jax.devices("cpu")[0]
    with jax.default_device(cpu):
        jflat = jax.device_put(flat, cpu)
        jrouter = jax.device_put(w_router, cpu)
        logits = jflat @ jrouter
        rw = jax.nn.softmax(logits, axis=-1)
        topw, topi = jax.lax.top_k(rw, TOPK)
        topw = topw / jax.numpy.sum(topw, axis=-1, keepdims=True)
    topw = np.asarray(topw)  # [T, K] f32
    topi = np.asarray(topi)  # [T, K] int32

    pairs_e = topi.ravel()  # expert of each (token, k) slot
    order = np.argsort(pairs_e, kind="stable")
    counts = np.bincount(pairs_e, minlength=E)
    starts = np.zeros(E + 1, np.int64)
    np.cumsum(counts, out=starts[1:])
    tok_by_e = [order[starts[e]:starts[e + 1]] // TOPK for e in range(E)]
    w_by_e = [topw.ravel()[order[starts[e]:starts[e + 1]]] for e in range(E)]

    R, V, chunks = _plan_overflow(counts)

    # --- build per-core inputs (fp16 / fp8, transposed activations) ---
    flatT16 = np.ascontiguousarray(flat.T.astype(np.float16))  # [H, T]
    w1s16 = np.asarray(w1_shared, dtype=np.float16)
    w1r16 = np.asarray(w1_routed, dtype=np.float16)

    def w2_prep(w2):
        w2 = np.asarray(w2, np.float32) * W2SC
        return (np.ascontiguousarray(w2[K8:]).astype(np.float16),
                np.ascontiguousarray(np.clip(w2[:K8], -240, 240)).astype(E4))

    w2s16, w2s8 = w2_prep(w2_shared)
    w2r = [w2_prep(np.asarray(w2_routed[e], np.float32)) for e in range(E)]

    zero_w1 = np.zeros((H, I), np.float16)
    zero_w2 = np.zeros((I - K8, H), np.float16)
    zero_w28 = np.zeros((K8, H), E4)

    in_maps = []
    for i in range(NCORES):
        xr_i = np.zeros((H, R), np.float16)
        nr = min(int(counts[i]), R)
        xr_i[:, :nr] = flatT16[:, tok_by_e[i][:nr]]
        m = {
            "xs": np.ascontiguousarray(flatT16[:, i * TS:(i + 1) * TS]),
            "xr": xr_i,
            "w1s": w1s16,
            "w2s": w2s16,
            "w28s": w2s8,
            "w1r": w1r16[i],
            "w2r": w2r[i][0],
            "w28r": w2r[i][1],
        }
        if V:
            if i < len(chunks):
                e, lo, hi = chunks[i]
                xv_i = np.zeros((H, V), np.float16)
                xv_i[:, :hi - lo] = flatT16[:, tok_by_e[e][lo:hi]]
                m["xv"] = xv_i
                m["w1v"] = w1r16[e]
                m["w2v"] = w2r[e][0]
                m["w28v"] = w2r[e][1]
            else:
                m["xv"] = np.zeros((H, V), np.float16)
                m["w1v"] = zero_w1
                m["w2v"] = zero_w2
                m["w28v"] = zero_w28
        in_maps.append(m)

    nc = _get_nc(R, V)
    try:
        res = run_bass_kernel_spmd(nc, in_maps, list(range(NCORES)),
                                   trace=TRACE)
    except Exception:
        # transient NRT/device hiccups have been observed to clear on retry
        res = run_bass_kernel_spmd(nc, in_maps, list(range(NCORES)),
                                   trace=TRACE)
    global LAST_RESULT
    LAST_RESULT = res

    # --- combine on host ---
    inv = np.float32(1.0 / W2SC)
    total = np.empty((T, H), np.float32)
    for i in range(NCORES):
        r = res.results[i]
        total[i * TS:(i + 1) * TS] = r["ys"].T + r["y8s"].astype(np.float32)
    total *= inv
    routed = np.zeros((T, H), np.float32)
    for e in range(E):
        ne = min(int(counts[e]), R)
        if ne:
            r = res.results[e]
            ye = (r["yr"][:, :ne].T
                  + r["y8r"][:ne].astype(np.float32)) * inv  # [ne, H]
            routed[tok_by_e[e][:ne]] += w_by_e[e][:ne, None] * ye
    for i, (e, lo, hi) in enumerate(chunks):
        r = res.results[i]
        yv = (r["yv"][:, :hi - lo].T
              + r["y8v"][:hi - lo].astype(np.float32)) * inv
        routed[tok_by_e[e][lo:hi]] += w_by_e[e][lo:hi, None] * yv
    total += routed
    return total.reshape(hidden_states.shape)
